# revision 5
# baseline (speedup 1.0000x reference)
"""Trainium2 Bass kernel for nn_BaselineSpanScorer (span-pair MLP scorer).

reference:
    xs        [32, 512, 1024] f32
    spans     [65536, 2] int   (begin/end token index within sequence)
    batch_ids [65536] int
    W1 [2048, 150], b1 [150], W2 [150, 17], b2 [17]
    out[n] = relu(concat(xs[b, s0], xs[b, s1]) @ W1 + b1) @ W2 + b2

Strategy (8 NeuronCores, data parallel with host routing):
  - Shard xs by batch: core c owns batches [4c, 4c+4) = 2048 token rows.
  - Route each span to the core owning its batch (host-side argsort).
  - Key algebraic factorization: for each token t precompute
        A[t] = xs[t] @ W1[:1024]      (begin-half table)
        G[t] = xs[t] @ W1[1024:]      (end-half table)
    then pre[n] = A[i0_n] + G[i1_n] + b1.  This reduces matmul work by
    the average span->token reuse factor (~8x) versus scoring spans
    directly.
  - Stage 1 (per core): A/G tables for its 2048 tokens via TensorE
    (fp16), written to a DRAM table of padded 256-elem rows.
  - Stage 2: transpose-mode dma_gather of the table rows for both span
    endpoints (features land on partitions), DVE add, ACT relu+bias,
    then a [256]x[17] TensorE contraction per 512-span tile, bias, and
    DMA out scores^T.
  - Host scatters per-core outputs back to the original span order.

Compute dtype fp16 (rel err ~1e-3 vs f32 reference), f32 output.
"""

import os

os.environ.setdefault("MYCRO_LOCAL_CACHE", "1")

import numpy as np

# ---------------- problem constants (hardcoded per spec) ----------------
B, T, D = 32, 512, 1024
N_SPANS = 65536
H, L = 150, 17
HP = 256                 # padded hidden (table row elems, fp16 -> 512B)
NCORES = 8
BPC = B // NCORES        # batches per core = 4
TC = BPC * T             # tokens per core = 2048
N_KB = D // 128          # K blocks in stage 1 = 8
N_TT = TC // 128         # token tiles in stage 1 = 16
SPAN_TILE = 512          # spans per stage-2 tile
W1N = 2 * H              # 300: stage-1 moving operand width (A | G)


def build_graph(m_pad: int):
    """Build the per-core SPMD Bass graph. m_pad = padded span count."""
    from concourse import bacc
    import concourse.mybir as mybir
    from concourse.tile import TileContext

    fp16 = mybir.dt.float16
    f32 = mybir.dt.float32
    i16 = mybir.dt.int16
    AF = mybir.ActivationFunctionType

    n_st = m_pad // SPAN_TILE
    nidx = 2 * SPAN_TILE             # indices per gather (A block | G block)
    idx_cols = nidx // 16            # 64

    nc = bacc.Bacc()

    xsT_d = nc.declare_dram_parameter("xsT", [D, TC], fp16, isOutput=False)
    wc_d = nc.declare_dram_parameter("wc", [128, N_KB * W1N], fp16, isOutput=False)
    w2p_d = nc.declare_dram_parameter("w2p", [128, 2 * L], fp16, isOutput=False)
    b1p_d = nc.declare_dram_parameter("b1p", [128, 2], f32, isOutput=False)
    b2p_d = nc.declare_dram_parameter("b2p", [L, 1], f32, isOutput=False)
    idx_d = nc.declare_dram_parameter("idx", [128, n_st * idx_cols], i16, isOutput=False)
    outT_d = nc.declare_dram_parameter("outT", [L, m_pad], f32, isOutput=True)

    with TileContext(nc) as tc:
        with (
            tc.tile_pool(name="const", bufs=1) as constp,
            tc.tile_pool(name="xst", bufs=1) as xstp,
            tc.tile_pool(name="dram", bufs=1, space="DRAM") as dramp,
            tc.tile_pool(name="ps1", bufs=4, space="PSUM") as ps1p,
            tc.tile_pool(name="tabt", bufs=4) as tabtp,
            tc.tile_pool(name="gat", bufs=3) as gatp,
            tc.tile_pool(name="act", bufs=3) as actp,
            tc.tile_pool(name="ps2", bufs=3, space="PSUM") as ps2p,
            tc.tile_pool(name="ot", bufs=3) as otp,
        ):
            # ---- constants ----
            wc_sb = constp.tile([128, N_KB * W1N], fp16)
            nc.sync.dma_start(out=wc_sb[:], in_=wc_d[:])
            w2p_sb = constp.tile([128, 2 * L], fp16)
            nc.sync.dma_start(out=w2p_sb[:], in_=w2p_d[:])
            b1p_sb = constp.tile([128, 2], f32)
            nc.sync.dma_start(out=b1p_sb[:], in_=b1p_d[:])
            b2p_sb = constp.tile([L, 1], f32)
            nc.sync.dma_start(out=b2p_sb[:], in_=b2p_d[:])
            idx_sb = constp.tile([128, n_st * idx_cols], i16)
            nc.sync.dma_start(out=idx_sb[:], in_=idx_d[:])

            # ---- stage 1: token tables A|G ----
            # xst layout: [128 (D within block), kb, token]
            xst_sb = xstp.tile([128, N_KB, TC], fp16)
            xsT_r = xsT_d.rearrange("(kb p) t -> p kb t", p=128)
            TB = 256  # tokens per load chunk (512B per partition line)
            for tb in range(TC // TB):
                nc.sync.dma_start(
                    out=xst_sb[:, :, tb * TB:(tb + 1) * TB],
                    in_=xsT_r[:, :, tb * TB:(tb + 1) * TB],
                )

            tab_t = dramp.tile([2 * TC, HP], fp16)  # A rows then G rows

            for tt in range(N_TT):
                ps = ps1p.tile([128, W1N], f32)
                for kb in range(N_KB):
                    nc.tensor.matmul(
                        ps[:],
                        xst_sb[:, kb, tt * 128:(tt + 1) * 128],
                        wc_sb[:, kb * W1N:(kb + 1) * W1N],
                        start=(kb == 0),
                        stop=(kb == N_KB - 1),
                    )
                ta = tabtp.tile([128, HP], fp16, tag="ta")
                tg = tabtp.tile([128, HP], fp16, tag="tg")
                nc.vector.memset(ta[:, H:HP], 0.0)
                nc.vector.memset(tg[:, H:HP], 0.0)
                nc.scalar.activation(ta[:, 0:H], ps[:, 0:H], AF.Copy)
                nc.vector.tensor_copy(tg[:, 0:H], ps[:, H:W1N])
                nc.sync.dma_start(
                    out=tab_t[tt * 128:(tt + 1) * 128, :], in_=ta[:]
                )
                nc.sync.dma_start(
                    out=tab_t[TC + tt * 128:TC + (tt + 1) * 128, :], in_=tg[:]
                )

            # all table rows must land before any gather reads them
            tc.strict_bb_all_engine_barrier()

            # ---- stage 2: gather + add + relu + W2 ----
            # HW limit: dma_gather handles at most 512 indices per op, so
            # gather the A-block and G-block of each span tile separately.
            half_cols = idx_cols // 2  # 32
            for st in range(n_st):
                ga = gatp.tile([128, 2, SPAN_TILE], fp16, tag="ga")
                nc.gpsimd.dma_gather(
                    ga[:],
                    tab_t[:, :],
                    idx_sb[:, st * idx_cols:st * idx_cols + half_cols],
                    SPAN_TILE,
                    SPAN_TILE,
                    elem_size=HP,
                    transpose=True,
                )
                gg = gatp.tile([128, 2, SPAN_TILE], fp16, tag="gg")
                nc.gpsimd.dma_gather(
                    gg[:],
                    tab_t[:, :],
                    idx_sb[:, st * idx_cols + half_cols:(st + 1) * idx_cols],
                    SPAN_TILE,
                    SPAN_TILE,
                    elem_size=HP,
                    transpose=True,
                )
                pre = actp.tile([128, 2, SPAN_TILE], fp16, tag="pre")
                nc.vector.tensor_add(pre[:], ga[:], gg[:])
                h = actp.tile([128, 2, SPAN_TILE], fp16, tag="h")
                for j in range(2):
                    nc.scalar.activation(
                        h[:, j, :], pre[:, j, :], AF.Relu, bias=b1p_sb[:, j:j + 1]
                    )
                ps2 = ps2p.tile([L, SPAN_TILE], f32)
                for j in range(2):
                    nc.tensor.matmul(
                        ps2[:],
                        w2p_sb[:, j * L:(j + 1) * L],
                        h[:, j, :],
                        start=(j == 0),
                        stop=(j == 1),
                    )
                ot = otp.tile([L, SPAN_TILE], f32)
                nc.scalar.activation(ot[:], ps2[:], AF.Identity, bias=b2p_sb[:])
                nc.sync.dma_start(
                    out=outT_d[:, st * SPAN_TILE:(st + 1) * SPAN_TILE], in_=ot[:]
                )

    return nc


def prep_inputs(xs, spans, batch_ids, W1, b1, W2, b2):
    """Host-side routing and layout. Returns (in_maps, per-core span ids, m_pad)."""
    xs = np.asarray(xs, dtype=np.float32)
    spans = np.asarray(spans).astype(np.int64)
    batch_ids = np.asarray(batch_ids).astype(np.int64)
    W1 = np.asarray(W1, dtype=np.float32)
    b1 = np.asarray(b1, dtype=np.float32)
    W2 = np.asarray(W2, dtype=np.float32)
    b2 = np.asarray(b2, dtype=np.float32)

    core = batch_ids // BPC
    local0 = (batch_ids % BPC) * T + spans[:, 0]
    local1 = (batch_ids % BPC) * T + spans[:, 1]

    order = np.argsort(core, kind="stable")
    counts = np.bincount(core, minlength=NCORES)
    offs = np.concatenate([[0], np.cumsum(counts)])
    m_pad = int(max(np.ceil(counts.max() / SPAN_TILE), 1) * SPAN_TILE)
    n_st = m_pad // SPAN_TILE

    # shared weights
    W1h = W1.astype(np.float16)
    wc = np.empty((128, N_KB * W1N), np.float16)
    for kb in range(N_KB):
        wc[:, kb * W1N:kb * W1N + H] = W1h[kb * 128:(kb + 1) * 128, :]
        wc[:, kb * W1N + H:(kb + 1) * W1N] = W1h[D + kb * 128:D + (kb + 1) * 128, :]
    W2pad = np.zeros((HP, L), np.float16)
    W2pad[:H] = W2.astype(np.float16)
    w2p = np.empty((128, 2 * L), np.float16)
    w2p[:, 0:L] = W2pad[0:128]
    w2p[:, L:2 * L] = W2pad[128:HP]
    b1pad = np.zeros((HP,), np.float32)
    b1pad[:H] = b1
    b1p = np.ascontiguousarray(b1pad.reshape(2, 128).T)
    b2p = np.ascontiguousarray(b2.reshape(L, 1))

    in_maps = []
    span_ids = []
    for c in range(NCORES):
        sel = order[offs[c]:offs[c + 1]]
        span_ids.append(sel)
        i0 = np.zeros(m_pad, np.int64)
        i1 = np.zeros(m_pad, np.int64)
        i0[:len(sel)] = local0[sel]
        i1[:len(sel)] = local1[sel]
        # combined gather index layout: per span-tile, 32 cols of A idx then
        # 32 cols of G idx (row-offset +TC); within a col 16 consecutive
        # positions down partitions, replicated 8x to fill 128 partitions.
        a_blk = i0.reshape(n_st, 32, 16)
        g_blk = (i1 + TC).reshape(n_st, 32, 16)
        blk = np.concatenate([a_blk, g_blk], axis=1)        # [st, 64, 16]
        arr16 = blk.transpose(2, 0, 1).reshape(16, n_st * 64)
        idxc = np.ascontiguousarray(np.tile(arr16, (8, 1)).astype(np.int16))

        xs_c = xs[c * BPC:(c + 1) * BPC].reshape(TC, D)
        xsT = np.ascontiguousarray(xs_c.T.astype(np.float16))
        in_maps.append({
            "xsT": xsT, "wc": wc, "w2p": w2p, "b1p": b1p, "b2p": b2p,
            "idx": idxc,
        })
    return in_maps, span_ids, m_pad


def _scatter_out(results, span_ids):
    out = np.empty((N_SPANS, L), np.float32)
    for c in range(NCORES):
        sel = span_ids[c]
        out[sel] = results[c]["outT"].T[:len(sel)]
    return out


def _install_ntff_shim():
    """Provide antenv.axon_hooks (missing on this image) so that
    run_bass_kernel_spmd(trace=True) can drive NTFF profiling via the
    axon .so. Only used by the profiling path."""
    import sys
    import types
    import ctypes
    import contextlib

    if "antenv.axon_hooks" in sys.modules:
        return
    import antenv

    holder = {"hook": None}
    mod = types.ModuleType("antenv.axon_hooks")
    mod.set_axon_ntff_profile_hook = lambda h: holder.__setitem__("hook", h)
    mod.get_axon_ntff_profile_hook = lambda: holder["hook"]
    sys.modules["antenv.axon_hooks"] = mod
    antenv.axon_hooks = mod

    so_path = "/opt/axon/libaxon_pjrt.so"
    try:
        lib = ctypes.CDLL(so_path)
    except OSError:
        return
    if not hasattr(lib, "axon_start_nrt_profile"):
        return
    lib.axon_start_nrt_profile.argtypes = [
        ctypes.POINTER(ctypes.c_int64),
        ctypes.c_size_t,
    ]
    lib.axon_start_nrt_profile.restype = ctypes.c_int64
    lib.axon_stop_nrt_profile.argtypes = [ctypes.c_char_p]
    lib.axon_stop_nrt_profile.restype = ctypes.c_int64

    @contextlib.contextmanager
    def _hook(output_dir, device_ids):
        import jax

        jax.devices()
        if device_ids:
            ids = (ctypes.c_int64 * len(device_ids))(*device_ids)
            rc = lib.axon_start_nrt_profile(ids, len(device_ids))
        else:
            rc = lib.axon_start_nrt_profile(None, 0)
        if rc != 0:
            raise RuntimeError(f"axon_start_nrt_profile rc={rc}")
        try:
            yield
        finally:
            n = lib.axon_stop_nrt_profile(str(output_dir).encode())
            print(f"profile: {n} file(s) written to {output_dir}")

    mod.set_axon_ntff_profile_hook(_hook)


def run(inputs: dict, trace: bool = False):
    """Run on the 8 NeuronCores. Returns (out, BassKernelResults)."""
    from concourse import bass_utils
    from concourse.bass_utils import run_bass_kernel_spmd

    if trace:
        _install_ntff_shim()
        # no artifact bucket in this environment
        bass_utils.upload_artifacts = lambda tmpdir: str(tmpdir)

    in_maps, span_ids, m_pad = prep_inputs(**inputs)
    nc = build_graph(m_pad)
    nc.finalize()
    res = run_bass_kernel_spmd(
        nc, in_maps, list(range(NCORES)), trace=trace
    )
    return _scatter_out(res.results, span_ids), res


def kernel(**inputs) -> np.ndarray:
    out, _ = run(inputs, trace=False)
    return out


# revision 8
# speedup vs baseline: 2.1396x; 2.1396x over previous
"""Trainium2 Bass kernel for nn_BaselineSpanScorer (span-pair MLP scorer).

reference:
    xs        [32, 512, 1024] f32
    spans     [65536, 2] int   (begin/end token index within sequence)
    batch_ids [65536] int
    W1 [2048, 150], b1 [150], W2 [150, 17], b2 [17]
    out[n] = relu(concat(xs[b, s0], xs[b, s1]) @ W1 + b1) @ W2 + b2

Strategy (8 NeuronCores, data parallel with host routing):
  - Shard xs by batch: core c owns batches [4c, 4c+4) = 2048 token rows.
  - Route each span to the core owning its batch (host-side argsort).
  - Key algebraic factorization: for each token t precompute
        A[t] = xs[t] @ W1[:1024]      (begin-half table)
        G[t] = xs[t] @ W1[1024:]      (end-half table)
    then pre[n] = A[i0_n] + G[i1_n] + b1.  This reduces matmul work by
    the average span->token reuse factor (~8x) versus scoring spans
    directly.
  - Stage 1 (per core): A/G tables for its 2048 tokens via TensorE
    (fp16), written to a DRAM table of padded 256-elem rows.
  - Stage 2: transpose-mode dma_gather of the table rows for both span
    endpoints (features land on partitions), DVE add, ACT relu+bias,
    then a [256]x[17] TensorE contraction per 512-span tile, bias, and
    DMA out scores^T.
  - Host scatters per-core outputs back to the original span order.

Compute dtype fp16 (rel err ~1e-3 vs f32 reference), f32 output.
"""

import os

os.environ.setdefault("MYCRO_LOCAL_CACHE", "1")

import numpy as np

# ---------------- problem constants (hardcoded per spec) ----------------
B, T, D = 32, 512, 1024
N_SPANS = 65536
H, L = 150, 17
HP = 256                 # padded hidden (table row elems, fp16 -> 512B)
NCORES = 8
BPC = B // NCORES        # batches per core = 4
TC = BPC * T             # tokens per core = 2048
N_KB = D // 128          # K blocks in stage 1 = 8
N_TT = TC // 128         # token tiles in stage 1 = 16
SPAN_TILE = 512          # spans per stage-2 tile
W1N = 2 * H              # 300: stage-1 moving operand width (A | G)


def build_graph(m_pad: int):
    """Build the per-core SPMD Bass graph. m_pad = padded span count."""
    from concourse import bacc
    import concourse.mybir as mybir
    from concourse.tile import TileContext

    fp16 = mybir.dt.float16
    f32 = mybir.dt.float32
    i16 = mybir.dt.int16
    AF = mybir.ActivationFunctionType

    n_st = m_pad // SPAN_TILE
    nidx = 2 * SPAN_TILE             # indices per gather (A block | G block)
    idx_cols = nidx // 16            # 64

    nc = bacc.Bacc(num_swdge_queues=4)

    xsT_d = nc.declare_dram_parameter("xsT", [D, TC], fp16, isOutput=False)
    wc_d = nc.declare_dram_parameter("wc", [128, N_KB * W1N], fp16, isOutput=False)
    w2p_d = nc.declare_dram_parameter("w2p", [128, 2 * L], fp16, isOutput=False)
    b1p_d = nc.declare_dram_parameter("b1p", [128, 2], f32, isOutput=False)
    b2p_d = nc.declare_dram_parameter("b2p", [L, 1], f32, isOutput=False)
    idx_d = nc.declare_dram_parameter("idx", [128, n_st * idx_cols], i16, isOutput=False)
    outT_d = nc.declare_dram_parameter("outT", [L, m_pad], f32, isOutput=True)

    with TileContext(nc) as tc:
        with (
            tc.tile_pool(name="const", bufs=1) as constp,
            tc.tile_pool(name="xst", bufs=1) as xstp,
            tc.tile_pool(name="dram", bufs=1, space="DRAM") as dramp,
            tc.tile_pool(name="ps1", bufs=4, space="PSUM") as ps1p,
            tc.tile_pool(name="tabt", bufs=4) as tabtp,
            tc.tile_pool(name="gat", bufs=8) as gatp,
            tc.tile_pool(name="act", bufs=3) as actp,
            tc.tile_pool(name="ps2", bufs=3, space="PSUM") as ps2p,
            tc.tile_pool(name="ot", bufs=3) as otp,
        ):
            # ---- constants ----
            wc_sb = constp.tile([128, N_KB * W1N], fp16)
            nc.sync.dma_start(out=wc_sb[:], in_=wc_d[:])
            w2p_sb = constp.tile([128, 2 * L], fp16)
            nc.sync.dma_start(out=w2p_sb[:], in_=w2p_d[:])
            b1p_sb = constp.tile([128, 2], f32)
            nc.sync.dma_start(out=b1p_sb[:], in_=b1p_d[:])
            b2p_sb = constp.tile([L, 1], f32)
            nc.sync.dma_start(out=b2p_sb[:], in_=b2p_d[:])
            idx_sb = constp.tile([128, n_st * idx_cols], i16)
            nc.sync.dma_start(out=idx_sb[:], in_=idx_d[:])

            # ---- stage 1: token tables A|G ----
            # xst layout: [128 (D within block), kb, token]
            xst_sb = xstp.tile([128, N_KB, TC], fp16)
            xsT_r = xsT_d.rearrange("(kb p) t -> p kb t", p=128)
            TB = 256  # tokens per load chunk (512B per partition line)
            for tb in range(TC // TB):
                nc.sync.dma_start(
                    out=xst_sb[:, :, tb * TB:(tb + 1) * TB],
                    in_=xsT_r[:, :, tb * TB:(tb + 1) * TB],
                )

            tab_t = dramp.tile([2 * TC, HP], fp16)  # A rows then G rows

            for tt in range(N_TT):
                ps = ps1p.tile([128, W1N], f32)
                for kb in range(N_KB):
                    nc.tensor.matmul(
                        ps[:],
                        xst_sb[:, kb, tt * 128:(tt + 1) * 128],
                        wc_sb[:, kb * W1N:(kb + 1) * W1N],
                        start=(kb == 0),
                        stop=(kb == N_KB - 1),
                    )
                ta = tabtp.tile([128, HP], fp16, tag="ta")
                tg = tabtp.tile([128, HP], fp16, tag="tg")
                nc.vector.memset(ta[:, H:HP], 0.0)
                nc.vector.memset(tg[:, H:HP], 0.0)
                nc.scalar.activation(ta[:, 0:H], ps[:, 0:H], AF.Copy)
                nc.vector.tensor_copy(tg[:, 0:H], ps[:, H:W1N])
                nc.sync.dma_start(
                    out=tab_t[tt * 128:(tt + 1) * 128, :], in_=ta[:]
                )
                nc.sync.dma_start(
                    out=tab_t[TC + tt * 128:TC + (tt + 1) * 128, :], in_=tg[:]
                )

            # all table rows must land before any gather reads them
            tc.strict_bb_all_engine_barrier()

            # ---- stage 2: gather + add + relu + W2 ----
            # HW limit: dma_gather handles at most 512 indices per op, so
            # gather the A-block and G-block of each span tile separately.
            half_cols = idx_cols // 2  # 32
            for st in range(n_st):
                ga = gatp.tile([128, 2, SPAN_TILE], fp16, tag="ga")
                nc.gpsimd.dma_gather(
                    ga[:],
                    tab_t[:, :],
                    idx_sb[:, st * idx_cols:st * idx_cols + half_cols],
                    SPAN_TILE,
                    SPAN_TILE,
                    elem_size=HP,
                    transpose=True,
                    queue_num=(2 * st) % 4,
                )
                gg = gatp.tile([128, 2, SPAN_TILE], fp16, tag="gg")
                nc.gpsimd.dma_gather(
                    gg[:],
                    tab_t[:, :],
                    idx_sb[:, st * idx_cols + half_cols:(st + 1) * idx_cols],
                    SPAN_TILE,
                    SPAN_TILE,
                    elem_size=HP,
                    transpose=True,
                    queue_num=(2 * st + 1) % 4,
                )
                pre = actp.tile([128, 2, SPAN_TILE], fp16, tag="pre")
                nc.vector.tensor_add(pre[:], ga[:], gg[:])
                h = actp.tile([128, 2, SPAN_TILE], fp16, tag="h")
                for j in range(2):
                    nc.scalar.activation(
                        h[:, j, :], pre[:, j, :], AF.Relu, bias=b1p_sb[:, j:j + 1]
                    )
                ps2 = ps2p.tile([L, SPAN_TILE], f32)
                for j in range(2):
                    nc.tensor.matmul(
                        ps2[:],
                        w2p_sb[:, j * L:(j + 1) * L],
                        h[:, j, :],
                        start=(j == 0),
                        stop=(j == 1),
                    )
                ot = otp.tile([L, SPAN_TILE], f32)
                nc.scalar.activation(ot[:], ps2[:], AF.Identity, bias=b2p_sb[:])
                nc.sync.dma_start(
                    out=outT_d[:, st * SPAN_TILE:(st + 1) * SPAN_TILE], in_=ot[:]
                )

    return nc


def prep_inputs(xs, spans, batch_ids, W1, b1, W2, b2):
    """Host-side routing and layout. Returns (in_maps, per-core span ids, m_pad)."""
    xs = np.asarray(xs, dtype=np.float32)
    spans = np.asarray(spans).astype(np.int64)
    batch_ids = np.asarray(batch_ids).astype(np.int64)
    W1 = np.asarray(W1, dtype=np.float32)
    b1 = np.asarray(b1, dtype=np.float32)
    W2 = np.asarray(W2, dtype=np.float32)
    b2 = np.asarray(b2, dtype=np.float32)

    core = batch_ids // BPC
    local0 = (batch_ids % BPC) * T + spans[:, 0]
    local1 = (batch_ids % BPC) * T + spans[:, 1]

    order = np.argsort(core, kind="stable")
    counts = np.bincount(core, minlength=NCORES)
    offs = np.concatenate([[0], np.cumsum(counts)])
    m_pad = int(max(np.ceil(counts.max() / SPAN_TILE), 1) * SPAN_TILE)
    n_st = m_pad // SPAN_TILE

    # shared weights
    W1h = W1.astype(np.float16)
    wc = np.empty((128, N_KB * W1N), np.float16)
    for kb in range(N_KB):
        wc[:, kb * W1N:kb * W1N + H] = W1h[kb * 128:(kb + 1) * 128, :]
        wc[:, kb * W1N + H:(kb + 1) * W1N] = W1h[D + kb * 128:D + (kb + 1) * 128, :]
    W2pad = np.zeros((HP, L), np.float16)
    W2pad[:H] = W2.astype(np.float16)
    w2p = np.empty((128, 2 * L), np.float16)
    w2p[:, 0:L] = W2pad[0:128]
    w2p[:, L:2 * L] = W2pad[128:HP]
    b1pad = np.zeros((HP,), np.float32)
    b1pad[:H] = b1
    b1p = np.ascontiguousarray(b1pad.reshape(2, 128).T)
    b2p = np.ascontiguousarray(b2.reshape(L, 1))

    in_maps = []
    span_ids = []
    for c in range(NCORES):
        sel = order[offs[c]:offs[c + 1]]
        span_ids.append(sel)
        i0 = np.zeros(m_pad, np.int64)
        i1 = np.zeros(m_pad, np.int64)
        i0[:len(sel)] = local0[sel]
        i1[:len(sel)] = local1[sel]
        # combined gather index layout: per span-tile, 32 cols of A idx then
        # 32 cols of G idx (row-offset +TC); within a col 16 consecutive
        # positions down partitions, replicated 8x to fill 128 partitions.
        a_blk = i0.reshape(n_st, 32, 16)
        g_blk = (i1 + TC).reshape(n_st, 32, 16)
        blk = np.concatenate([a_blk, g_blk], axis=1)        # [st, 64, 16]
        arr16 = blk.transpose(2, 0, 1).reshape(16, n_st * 64)
        idxc = np.ascontiguousarray(np.tile(arr16, (8, 1)).astype(np.int16))

        xs_c = xs[c * BPC:(c + 1) * BPC].reshape(TC, D)
        xsT = np.ascontiguousarray(xs_c.T.astype(np.float16))
        in_maps.append({
            "xsT": xsT, "wc": wc, "w2p": w2p, "b1p": b1p, "b2p": b2p,
            "idx": idxc,
        })
    return in_maps, span_ids, m_pad


def _scatter_out(results, span_ids):
    out = np.empty((N_SPANS, L), np.float32)
    for c in range(NCORES):
        sel = span_ids[c]
        out[sel] = results[c]["outT"].T[:len(sel)]
    return out


def _install_ntff_shim():
    """Provide antenv.axon_hooks (missing on this image) so that
    run_bass_kernel_spmd(trace=True) can drive NTFF profiling via the
    axon .so. Only used by the profiling path."""
    import sys
    import types
    import ctypes
    import contextlib

    if "antenv.axon_hooks" in sys.modules:
        return
    import antenv

    holder = {"hook": None}
    mod = types.ModuleType("antenv.axon_hooks")
    mod.set_axon_ntff_profile_hook = lambda h: holder.__setitem__("hook", h)
    mod.get_axon_ntff_profile_hook = lambda: holder["hook"]
    sys.modules["antenv.axon_hooks"] = mod
    antenv.axon_hooks = mod

    so_path = "/opt/axon/libaxon_pjrt.so"
    try:
        lib = ctypes.CDLL(so_path)
    except OSError:
        return
    if not hasattr(lib, "axon_start_nrt_profile"):
        return
    lib.axon_start_nrt_profile.argtypes = [
        ctypes.POINTER(ctypes.c_int64),
        ctypes.c_size_t,
    ]
    lib.axon_start_nrt_profile.restype = ctypes.c_int64
    lib.axon_stop_nrt_profile.argtypes = [ctypes.c_char_p]
    lib.axon_stop_nrt_profile.restype = ctypes.c_int64

    @contextlib.contextmanager
    def _hook(output_dir, device_ids):
        import jax

        jax.devices()
        if device_ids:
            ids = (ctypes.c_int64 * len(device_ids))(*device_ids)
            rc = lib.axon_start_nrt_profile(ids, len(device_ids))
        else:
            rc = lib.axon_start_nrt_profile(None, 0)
        if rc != 0:
            raise RuntimeError(f"axon_start_nrt_profile rc={rc}")
        try:
            yield
        finally:
            n = lib.axon_stop_nrt_profile(str(output_dir).encode())
            print(f"profile: {n} file(s) written to {output_dir}")

    mod.set_axon_ntff_profile_hook(_hook)


def run(inputs: dict, trace: bool = False):
    """Run on the 8 NeuronCores. Returns (out, BassKernelResults)."""
    from concourse import bass_utils
    from concourse.bass_utils import run_bass_kernel_spmd

    if trace:
        _install_ntff_shim()
        # no artifact bucket in this environment
        bass_utils.upload_artifacts = lambda tmpdir: str(tmpdir)

    in_maps, span_ids, m_pad = prep_inputs(**inputs)
    nc = build_graph(m_pad)
    nc.finalize()
    res = run_bass_kernel_spmd(
        nc, in_maps, list(range(NCORES)), trace=trace
    )
    return _scatter_out(res.results, span_ids), res


def kernel(**inputs) -> np.ndarray:
    out, _ = run(inputs, trace=False)
    return out
